# revision 6
# baseline (speedup 1.0000x reference)
"""Bilinear pooling kernel for 8 Trainium2 NeuronCores (Bass/Tile).

Computes out[b,n,v,o] = sum_{d,e} node[b,n,d] * veh[b,v,e] * W[o, d*E+e] + bias[o]
for B=16, N=64, V=16, D=E=128, O=256.

Strategy: tensor-shard over the output dim O (32 channels per core).
Two matmul stages pipelined in two o-halves:
  Stage A:  U[d, (o,b,v)] = sum_e W3[o,d,e] * veh[b,v,e]
            per o: lhsT = W3[o].T [e,d], rhs = vehT [e, (b,v)=256];
            psum groups of 4 channels, evacuated to bf16 U by vector+scalar.
  Stage B:  out[b][n, (o,v)] = sum_d node[b,n,d] * U[d, o-half, b, v]
            2 batches per psum tile via PE column tiling (the two 64-wide
            column groups stream concurrently), 4 batches per psum bank.
Inputs stream in 8 graduated chunks across the 3 DMA queues (sync/scalar
HWDGE + gpsimd SWDGE, ~80 GB/s each) ordered by consumption deadline.
Outputs are cast to bf16 and DMAd as contiguous 128KB tiles spread over
all 3 queues; the host adds the bias in f32 during the unshard.
"""

import sys

import numpy as np

sys.path.insert(0, "/opt/trn_rl_repo")

B, N, V = 16, 64, 16
D = 128
E = 128
O = 256
NCORES = 8
OS = O // NCORES  # 32 output channels per core

WARM = 6  # PE-clock warmup matmuls while the first input chunks stream in
TAIL_MM = 10  # post-compute dummy matmuls keep the PE clock high through
#               the NEFF's semaphore-reset epilogue (Tensor is its straggler)

_nc_cache = {}


def _build():
    from contextlib import ExitStack

    import concourse.tile as tile
    from concourse import bacc, mybir

    f32 = mybir.dt.float32
    bf16 = mybir.dt.bfloat16

    nc = bacc.Bacc("TRN2", target_bir_lowering=False)
    vehT_d = nc.dram_tensor("vehT", [E, B * V], bf16, kind="ExternalInput")
    wg_d = [
        nc.dram_tensor(f"wg{g}", [E, 4 * D], bf16, kind="ExternalInput")
        for g in range(8)
    ]
    nodeT_d = nc.dram_tensor("nodeT", [D, B * N], bf16, kind="ExternalInput")
    # out tiles: t = h*4+q covers batches 4q..4q+3 of o-half h;
    # [128, 512] per tile: partition p=(b%2)*64+n, free = (j=(b%4)//2, ch, v)
    out_d = nc.dram_tensor("out", [8, 128, 512], bf16, kind="ExternalOutput")

    with ExitStack() as ctx:
        tc = ctx.enter_context(tile.TileContext(nc))
        const = ctx.enter_context(tc.tile_pool(name="const", bufs=1))
        upool = ctx.enter_context(tc.tile_pool(name="u", bufs=2))
        psA = ctx.enter_context(tc.tile_pool(name="psA", bufs=2, space="PSUM"))
        psB = ctx.enter_context(tc.tile_pool(name="psB", bufs=4, space="PSUM"))
        outp = ctx.enter_context(tc.tile_pool(name="outp", bufs=4))

        # ---- input DMAs: 3 parallel queues, chunks ordered by deadline ----
        vehT_t = const.tile([E, B * V], bf16)
        nc.gpsimd.dma_start(vehT_t[:], vehT_d[:])  # needed first (stage A rhs)
        wg = [None] * 8
        qeng = {0: nc.sync, 1: nc.scalar, 2: nc.sync, 3: nc.scalar,
                4: nc.sync, 5: nc.sync, 6: nc.scalar, 7: nc.gpsimd}
        # per-queue issue order = consumption order
        for g in (0, 1, 2, 3):  # first on their queues
            wg[g] = const.tile([E, 4 * D], bf16, name=f"wg{g}t")
            qeng[g].dma_start(wg[g][:], wg_d[g][:])
        nodeT_t = const.tile([D, B * N], bf16)
        nc.gpsimd.dma_start(nodeT_t[:], nodeT_d[:])  # needed at stage B0
        for g in (4, 5, 6, 7):
            wg[g] = const.tile([E, 4 * D], bf16, name=f"wg{g}t")
            qeng[g].dma_start(wg[g][:], wg_d[g][:])
        nodeT = nodeT_t[:]
        vehT = vehT_t[:]

        # ---- PE warmup on a zeroed tile (vector memset is its first op) ----
        warm = const.tile([E, B * V], bf16)
        nc.vector.memset(warm[:], 0)
        wps = psA.tile([D, 4, B * V], f32, tag="pa")
        for i in range(WARM):
            nc.tensor.matmul(wps[:, i % 4], warm[:, 0:D], warm[:], start=True, stop=True)

        U = [
            upool.tile([D, 16, B, V], bf16, tag="U", name=f"U{h}") for h in range(2)
        ]

        def stageA(g):
            # 4 channels (o = 4g .. 4g+3) -> psum [128, 4, 256] -> U[h]
            pa = psA.tile([D, 4, B * V], f32, tag="pa")
            for i in range(4):
                nc.tensor.matmul(
                    pa[:, i], wg[g][:, i * D : (i + 1) * D], vehT, start=True, stop=True
                )
            h, gl = divmod(g, 4)
            dst = U[h]
            nc.vector.tensor_copy(dst[:, 4 * gl : 4 * gl + 2, :, :], pa[:, 0:2])
            nc.scalar.copy(dst[:, 4 * gl + 2 : 4 * gl + 4, :, :], pa[:, 2:4])

        def stageB(h, q, ceng, deng, split=False):
            # batches 4q..4q+3 of o-half h -> psum [128, 2, 256] -> out tile
            pb = psB.tile([N * 2, 2, 256], f32, tag="pb")
            for j in range(2):
                for pbi in range(2):
                    b = 4 * q + 2 * j + pbi
                    nc.tensor.matmul(
                        pb[64 * pbi : 64 * (pbi + 1), j],
                        nodeT[:, b * N : (b + 1) * N],
                        U[h][:, :, b, :],
                        start=True,
                        stop=True,
                    )
            ob = outp.tile([128, 512], bf16)
            if not split:
                ceng(ob[:], pb[:])
                deng.dma_start(out_d[4 * h + q], ob[:])
            else:
                # final tile: halve copy+DMA latency via two queues
                nc.vector.tensor_copy(ob[:, 0:256], pb[:, 0])
                nc.scalar.copy(ob[:, 256:512], pb[:, 1])
                nc.sync.dma_start(out_d[4 * h + q, :, 0:256], ob[:, 0:256])
                nc.gpsimd.dma_start(out_d[4 * h + q, :, 256:512], ob[:, 256:512])

        vcp = nc.vector.tensor_copy
        scp = nc.scalar.copy
        # pipeline: A g0..g4 (g4 hides the U-copy latency before B0),
        # B(h0), A g5..g7, B(h1)
        for g in range(5):
            stageA(g)
        stageB(0, 0, vcp, nc.sync)
        stageB(0, 1, scp, nc.scalar)
        stageB(0, 2, vcp, nc.gpsimd)
        stageB(0, 3, scp, nc.sync)
        for g in range(5, 8):
            stageA(g)
        stageB(1, 0, vcp, nc.scalar)
        stageB(1, 1, scp, nc.gpsimd)
        stageB(1, 2, vcp, nc.sync)
        stageB(1, 3, None, None, split=True)

        # keep the PE clock gate open through the epilogue
        tps = psA.tile([D, 4, B * V], f32, tag="pa")
        for i in range(TAIL_MM):
            nc.tensor.matmul(tps[:, i % 4], warm[:, 0:D], warm[:], start=True, stop=True)

    nc.compile()
    return nc


def _get_nc():
    if "nc" not in _nc_cache:
        _nc_cache["nc"] = _build()
    return _nc_cache["nc"]


def _prep_inputs(node_embed, veh_fea, W, b):
    import ml_dtypes

    def cast(x):
        return np.ascontiguousarray(x.astype(ml_dtypes.bfloat16))

    node_embed = np.asarray(node_embed, dtype=np.float32)
    veh_fea = np.asarray(veh_fea, dtype=np.float32)
    W = np.asarray(W, dtype=np.float32)

    nodeT = cast(node_embed.transpose(2, 0, 1).reshape(D, B * N))
    vehT = cast(veh_fea.transpose(2, 0, 1).reshape(E, B * V))
    W3 = W.reshape(O, D, E)

    in_maps = []
    for c in range(NCORES):
        # [E, o_local, D] channel-major weights for this core's O-shard
        wtc = W3[c * OS : (c + 1) * OS].transpose(2, 0, 1).reshape(E, OS * D)
        m = {"vehT": vehT, "nodeT": nodeT}
        for g in range(8):
            m[f"wg{g}"] = cast(wtc[:, g * 4 * D : (g + 1) * 4 * D])
        in_maps.append(m)
    return in_maps


def run(node_embed, veh_fea, W, b, trace=False):
    from concourse.bass_utils import run_bass_kernel_spmd

    nc = _get_nc()
    in_maps = _prep_inputs(node_embed, veh_fea, W, b)
    res = run_bass_kernel_spmd(nc, in_maps, list(range(NCORES)), trace=trace)
    outs = []
    for r in res.results:
        # [8, 128, 512] -> [h, q, pb, n, j, ch, v] -> [b, n, v, (h,ch)]
        arr = np.asarray(r["out"]).astype(np.float32)
        arr = arr.reshape(2, 4, 2, 64, 2, 16, 16)
        arr = arr.transpose(1, 4, 2, 3, 6, 0, 5).reshape(B, N, V, OS)
        outs.append(arr)
    full = np.concatenate(outs, axis=3) + np.asarray(b, np.float32)
    return np.ascontiguousarray(full, dtype=np.float32), res


def kernel(node_embed, veh_fea, W, b):
    return run(node_embed, veh_fea, W, b)[0]
